# revision 12
# baseline (speedup 1.0000x reference)
"""Trainium2 Bass kernel for nn_Alpha2Assoc: 3-layer alpha compositing
with exclusive cumprod over D=32 planes.

Math per pixel (d = plane index, vectors over d):
    vis_l = excl_cumprod(1 - a_l);  out_l = vis_l * occ_{l-1};
    occ_l = 1 - vis_l;              a_{l+1} = a_l * occ_l
Output: stack([out_1, out_2, out_3], axis=2) -> [B, D, 3, H, W]

v2 design (fp16 I/O + two-path hybrid, engine-balanced):
  - All HBM I/O in fp16 (host converts): halves DMA traffic vs f32.
    Input clamped to <= 1-2^-11 host-side so ln(1-a) stays finite.
  - Path A ((b,d) on partitions, HA rows/core): cumprod in log space.
    Ln on ScalarE (affine absorbs 1-a), exclusive cumsum via TensorE
    f32r matmul with block-diag strictly-triangular 0/1 matrix,
    Exp(PSUM)->fp16 on ScalarE.  occ/a-recursion + out-muls on DVE
    (fp16: tensor_scalar 4x, tensor_tensor 2x).  ScalarE ~11.2us/tile.
  - Path B (transposed: pixel-sites on partitions, d contiguous in
    free dim, HB rows/core): DVE-only.  Segmented inclusive cumprod
    via tensor_tensor_scan(state = max(mask,state)*t) where mask=1 at
    d=0 cols resets each 32-group; scan writes at +1 col offset so
    consumers read the *exclusive* product at aligned offsets; a
    strided memset sets group-boundary cols to 1.  ~20.8us/tile, all
    on the otherwise-idle DVE.
  - Split HA/HB balances ScalarE (A only) against DVE (A muls + all
    of B).  B sub-stages (one per layer) are drip-fed between A
    pipeline stages.
"""

import math
from collections import deque

import numpy as np

import concourse.bass as bass
import concourse.tile as tile
from concourse import bacc, mybir
from concourse._compat import with_exitstack
from concourse.bass_utils import run_bass_kernel_spmd

# --- Pin Ln/Exp to the one table set containing both -------------------
_orig_get_activation_tables = bacc.get_activation_tables


def _pinned_get_activation_tables(arch):
    tables = _orig_get_activation_tables(arch)
    keep = {mybir.ActivationFunctionType.Ln, mybir.ActivationFunctionType.Exp}
    return {
        name: (fns if name == "natural_log_exp_and_others" else fns - keep)
        for name, fns in tables.items()
    }


bacc.get_activation_tables = _pinned_get_activation_tables

# --- Enable walrus LDWEIGHTS dedup (tri stationary identical across MMs)
import concourse.bass_utils as _bu

_orig_run_command = _bu.run_command


def _ldwopt_run_command(cmd, **kw):
    cmd = [c.replace("--enable-ldw-opt=false", "--enable-ldw-opt=true")
           if isinstance(c, str) else c for c in cmd]
    return _orig_run_command(cmd, **kw)


_bu.run_command = _ldwopt_run_command

B, D, H, W = 4, 32, 512, 512
P = 128
N_CORES = 8
H_SH = H // N_CORES                 # 64 rows per core
HA = 51                             # rows/core via log-space path A
HB = H_SH - HA                      # rows/core via DVE-scan path B
CA = HA * W                         # A cols per partition ((b,d) layout)
SITES_B = B * HB * W // P           # pixel-sites per partition (B layout)
CB = SITES_B * D                    # B cols per partition
TILE_N = 2048
MM_CHUNK = 512                      # f32r moving-operand max per matmul
PS_N = 2048                         # PSUM chunk (4 banks)


def _tiles(total, n):
    offs, sizes, o = [], [], 0
    while o < total:
        s = min(n, total - o)
        offs.append(o)
        sizes.append(s)
        o += s
    return list(zip(offs, sizes))


A_TILES = _tiles(CA, TILE_N)
B_TILES = _tiles(CB, TILE_N)

F16 = mybir.dt.float16
F32 = mybir.dt.float32
F32R = mybir.dt.float32r
AF = mybir.ActivationFunctionType
OP = mybir.AluOpType

_COMPILED = {}


def _tri_matrix() -> np.ndarray:
    """lhsT[k, m] = 1 iff same b-block and d_k < d_m (exclusive cumsum)."""
    k = np.arange(P)
    m = np.arange(P)
    same_b = (k[:, None] // D) == (m[None, :] // D)
    lower = (k[:, None] % D) < (m[None, :] % D)
    return (same_b & lower).astype(np.float32)


@with_exitstack
def _alpha_kernel(ctx, tc, aps):
    nc = tc.nc
    const_pool = ctx.enter_context(tc.tile_pool(name="const", bufs=1))
    # Warm the Ln/Exp act-table immediately (table load ~1.3us) so the
    # first real Ln doesn't serialize behind it.
    dummy = const_pool.tile([P, 8], F32)
    nc.vector.memset(dummy[:], 0.5)
    dummy_o = const_pool.tile([P, 8], F32)
    nc.scalar.activation(dummy_o[:], dummy[:], AF.Ln, bias=1.0, scale=-1.0)
    tri = const_pool.tile([P, P], F32R)
    msk = const_pool.tile([P, TILE_N], F16)

    # Path-A pools
    aA_pool = ctx.enter_context(tc.tile_pool(name="aA", bufs=5))
    u_pool = ctx.enter_context(tc.tile_pool(name="u", bufs=3))
    v_pool = ctx.enter_context(tc.tile_pool(name="v", bufs=8))
    occ_pool = ctx.enter_context(tc.tile_pool(name="occ", bufs=5))
    s_pool = ctx.enter_context(tc.tile_pool(name="s", bufs=5))
    o_pool = ctx.enter_context(tc.tile_pool(name="o", bufs=3))
    psum_pool = ctx.enter_context(tc.tile_pool(name="ps", bufs=2, space="PSUM"))
    # Path-B pools
    aB_pool = ctx.enter_context(tc.tile_pool(name="aB", bufs=2))
    t_pool = ctx.enter_context(tc.tile_pool(name="t", bufs=2))
    vb_pool = ctx.enter_context(tc.tile_pool(name="vb", bufs=3))
    occB_pool = ctx.enter_context(tc.tile_pool(name="occB", bufs=4))
    sB_pool = ctx.enter_context(tc.tile_pool(name="sB", bufs=4))
    oB_pool = ctx.enter_context(tc.tile_pool(name="oB", bufs=3))

    def cumsum_exp(u, vis, n):
        # f32r single-pass matmul; PSUM in PS_N units so matmul->exp
        # pipelines across banks.
        off = 0
        while off < n:
            psn = min(PS_N, n - off)
            ps = psum_pool.tile([P, psn], F32, tag="ps")
            for j0 in range(0, psn, MM_CHUNK):
                mc = min(MM_CHUNK, psn - j0)
                nc.tensor.matmul(
                    ps[:, bass.ds(j0, mc)],
                    tri[:],
                    u[:, bass.ds(off + j0, mc)],
                    start=True,
                    stop=True,
                )
            nc.scalar.activation(
                vis[:, bass.ds(off, psn)], ps[:, bass.ds(0, psn)], AF.Exp
            )
            off += psn

    # ---- Path A stages (pipelined across tiles) ----
    stA = {}
    stA_pre = {}

    def prefetch_a(i):
        if i < len(A_TILES) and i not in stA_pre:
            off, n = A_TILES[i]
            a1 = aA_pool.tile([P, n], F16, tag="aA")
            nc.sync.dma_start(a1[:], aps["aA"][:, bass.ds(off, n)])
            stA_pre[i] = a1

    def stage_a1(i):
        off, n = A_TILES[i]
        sl = bass.ds(off, n)
        a1 = stA_pre.pop(i)
        u1 = u_pool.tile([P, n], F32R, tag="u")
        nc.scalar.activation(u1[:], a1[:], AF.Ln, bias=1.0, scale=-1.0)
        v1 = v_pool.tile([P, n], F16, tag="v")
        cumsum_exp(u1, v1, n)
        nc.sync.dma_start(aps["o1A"][:, sl], v1[:])
        stA[i] = {"a1": a1, "v1": v1}

    def stage_a2v(i):
        # Vector half, issued one iteration before the ScalarE consumer so
        # B-scan jitter in the DVE queue can't stall Ln2.
        off, n = A_TILES[i]
        d = stA[i]
        occ1 = occ_pool.tile([P, n], F16, tag="occ")
        nc.vector.tensor_scalar(occ1[:], d["v1"][:], -1.0, 1.0, OP.mult, OP.add)
        a2 = s_pool.tile([P, n], F16, tag="s")
        nc.vector.tensor_mul(a2[:], d["a1"][:], occ1[:])
        d.update({"occ1": occ1, "a2": a2})

    def stage_a2s(i):
        off, n = A_TILES[i]
        d = stA[i]
        u2 = u_pool.tile([P, n], F32R, tag="u")
        nc.scalar.activation(u2[:], d["a2"][:], AF.Ln, bias=1.0, scale=-1.0)
        v2 = v_pool.tile([P, n], F16, tag="v")
        cumsum_exp(u2, v2, n)
        d["v2"] = v2

    def stage_a3v(i):
        off, n = A_TILES[i]
        sl = bass.ds(off, n)
        d = stA[i]
        o2 = o_pool.tile([P, n], F16, tag="o")
        nc.vector.tensor_mul(o2[:], d["v2"][:], d["occ1"][:])
        nc.sync.dma_start(aps["o2A"][:, sl], o2[:])
        occ2 = occ_pool.tile([P, n], F16, tag="occ")
        nc.vector.tensor_scalar(occ2[:], d["v2"][:], -1.0, 1.0, OP.mult, OP.add)
        a3 = s_pool.tile([P, n], F16, tag="s")
        nc.vector.tensor_mul(a3[:], d["a2"][:], occ2[:])
        d.update({"occ2": occ2, "a3": a3})

    def stage_a3s(i):
        off, n = A_TILES[i]
        d = stA[i]
        u3 = u_pool.tile([P, n], F32R, tag="u")
        nc.scalar.activation(u3[:], d["a3"][:], AF.Ln, bias=1.0, scale=-1.0)
        v3 = v_pool.tile([P, n], F16, tag="v")
        cumsum_exp(u3, v3, n)
        d["v3"] = v3

    def stage_a4(i):
        off, n = A_TILES[i]
        sl = bass.ds(off, n)
        d = stA[i]
        o3 = o_pool.tile([P, n], F16, tag="o")
        nc.vector.tensor_mul(o3[:], d["v3"][:], d["occ2"][:])
        nc.sync.dma_start(aps["o3A"][:, sl], o3[:])
        del stA[i]

    # ---- Path B ops (DVE-only), emitted as fine-grained closures so they
    # slot between A stages without head-of-line blocking the A-path DVE
    # ops that ScalarE waits on. Input DMAs are prefetched ~1 tile ahead.
    stB = {}

    def scan_layer(tag, j, t, n):
        """inclusive segmented cumprod of t -> vE ([P, n]: exclusive
        product per col after boundary memset; group cols == 1)."""
        vbuf = vb_pool.tile([P, n + 32], F16, tag="vb")
        nc.vector.tensor_tensor_scan(
            vbuf[:, bass.ds(1, n)], msk[:, bass.ds(0, n)], t[:],
            0.0, OP.max, OP.mult,
        )
        ve = vbuf[:, bass.ds(0, n)]
        gcols = ve.rearrange("p (g d) -> p g d", d=32)[:, :, bass.ds(0, 1)]
        nc.vector.memset(gcols, 1.0)
        stB[j][tag] = ve

    def b_ops_for_tile(j):
        off, n = B_TILES[j]
        sl = bass.ds(off, n)
        d = stB.setdefault(j, {})

        def dma_in():
            a1 = aB_pool.tile([P, n], F16, tag="aB")
            nc.sync.dma_start(a1[:], aps["aB"][:, sl])
            d["a1"] = a1

        def t1():
            t = t_pool.tile([P, n], F16, tag="t")
            nc.vector.tensor_scalar(t[:], d["a1"][:], -1.0, 1.0,
                                    OP.mult, OP.add)
            d["t"] = t

        def occ_a(v_tag, a_tag, occ_tag, a_out_tag):
            occ = occB_pool.tile([P, n], F16, tag="occB")
            nc.vector.tensor_scalar(occ[:], d[v_tag], -1.0, 1.0,
                                    OP.mult, OP.add)
            a_n = sB_pool.tile([P, n], F16, tag="sB")
            nc.vector.tensor_mul(a_n[:], d[a_tag][:], occ[:])
            d[occ_tag], d[a_out_tag] = occ, a_n

        def t_next(a_tag):
            t = t_pool.tile([P, n], F16, tag="t")
            nc.vector.tensor_scalar(t[:], d[a_tag][:], -1.0, 1.0,
                                    OP.mult, OP.add)
            d["t"] = t

        def out_mul(v_tag, occ_tag, out_name):
            o = oB_pool.tile([P, n], F16, tag="oB")
            nc.vector.tensor_mul(o[:], d[v_tag], d[occ_tag][:])
            nc.sync.dma_start(aps[out_name][:, sl], o[:])

        return [
            dma_in,
            t1,
            lambda: scan_layer("ve1", j, d["t"], n),
            lambda: nc.sync.dma_start(aps["o1B"][:, sl], d["ve1"]),
            lambda: occ_a("ve1", "a1", "occ1", "a2"),
            lambda: t_next("a2"),
            lambda: scan_layer("ve2", j, d["t"], n),
            lambda: out_mul("ve2", "occ1", "o2B"),
            lambda: occ_a("ve2", "a2", "occ2", "a3"),
            lambda: t_next("a3"),
            lambda: scan_layer("ve3", j, d["t"], n),
            lambda: out_mul("ve3", "occ2", "o3B"),
            lambda: stB.pop(j),
        ]

    # flatten, hoisting each tile's dma_in 4 ops before the end of the
    # previous tile's ops (prefetch)
    b_flat = []
    for j in range(len(B_TILES)):
        ops = b_ops_for_tile(j)
        if j == 0:
            b_flat += ops
        else:
            dma, rest = ops[0], ops[1:]
            b_flat = b_flat[:-3] + [dma] + b_flat[-3:] + rest
    b_work = deque(b_flat)

    # ---- interleaved schedule ----
    # Stage offsets give every cross-engine dependency >= 1 full iteration
    # of slack: v1(i)->a2v at i+1 ->Ln2 at i+2; v2(i)@i+2 ->a3v at i+3
    # ->Ln3 at i+4; out3 at i+5.
    nA = len(A_TILES)
    n_iters = nA + 5
    quota = len(b_work) / (2 * (n_iters - 1))
    state = {"credit": 0.0}

    def drain_b(mult=1.0):
        state["credit"] += quota * mult
        while b_work and state["credit"] >= 1.0:
            b_work.popleft()()
            state["credit"] -= 1.0

    prefetch_a(0)
    drain_b(1.0)  # issues the aB(0) input DMA early
    nc.sync.dma_start(tri[:], aps["tri"][:, :])
    nc.sync.dma_start(msk[:], aps["msk"][:, :])
    drain_b(2.0)
    for k in range(n_iters):
        prefetch_a(k + 1)
        if k < nA:
            stage_a1(k)
        if 1 <= k - 1 + 1 and k - 1 < nA and k >= 1:
            stage_a2v(k - 1)
        if 2 <= k and k - 2 < nA:
            stage_a2s(k - 2)
        if 3 <= k and k - 3 < nA:
            stage_a3v(k - 3)
        drain_b()
        if 4 <= k and k - 4 < nA:
            stage_a3s(k - 4)
        if 5 <= k and k - 5 < nA:
            stage_a4(k - 5)
        drain_b()
    while b_work:
        b_work.popleft()()


def _build():
    nc = bacc.Bacc("TRN2", target_bir_lowering=False, debug=False,
                   num_devices=N_CORES)
    aps = {
        "aA": nc.dram_tensor("aA", [P, CA], F16, kind="ExternalInput").ap(),
        "aB": nc.dram_tensor("aB", [P, CB], F16, kind="ExternalInput").ap(),
        "tri": nc.dram_tensor("tri", [P, P], F32R, kind="ExternalInput").ap(),
        "msk": nc.dram_tensor("msk", [P, TILE_N], F16, kind="ExternalInput").ap(),
    }
    for name, cols in [("o1A", CA), ("o2A", CA), ("o3A", CA),
                       ("o1B", CB), ("o2B", CB), ("o3B", CB)]:
        aps[name] = nc.dram_tensor(name, [P, cols], F16,
                                   kind="ExternalOutput").ap()
    with tile.TileContext(nc) as tc:
        _alpha_kernel(tc, aps, )
    nc.compile()
    return nc


def _get_nc():
    if "nc" not in _COMPILED:
        _COMPILED["nc"] = _build()
    return _COMPILED["nc"]


def _run(alpha_imgs: np.ndarray, trace: bool = False):
    nc = _get_nc()
    tri = _tri_matrix()
    msk = np.zeros((P, TILE_N), np.float16)
    msk[:, ::32] = 1.0
    # fp16 with clamp so ln(1-a) stays finite (a==1.0 after fp16 rounding
    # would give -inf and 0*inf=NaN in the cumsum matmul)
    a16 = np.minimum(np.asarray(alpha_imgs, dtype=np.float16),
                     np.float16(1.0 - 2.0 ** -11))
    in_maps = []
    for c in range(N_CORES):
        blk = a16[:, :, 0, c * H_SH:(c + 1) * H_SH, :]   # [B, D, 64, W]
        aA = np.ascontiguousarray(blk[:, :, :HA, :]).reshape(P, CA)
        aB = np.ascontiguousarray(
            blk[:, :, HA:, :].transpose(0, 2, 3, 1)       # [B, HB, W, D]
        ).reshape(P, CB)
        in_maps.append({"aA": aA, "aB": aB, "tri": tri, "msk": msk})
    res = None
    backoffs = [3.0, 10.0, 30.0, 60.0]
    for attempt in range(len(backoffs) + 1):
        try:
            res = run_bass_kernel_spmd(
                nc, in_maps, core_ids=list(range(N_CORES)), trace=trace
            )
            break
        except Exception:
            if attempt == len(backoffs):
                raise
            import time

            time.sleep(backoffs[attempt])
    out = np.empty((B, D, 3, H, W), dtype=np.float32)
    for c in range(N_CORES):
        r = res.results[c]
        rows = slice(c * H_SH, c * H_SH + HA)
        rowsB = slice(c * H_SH + HA, (c + 1) * H_SH)
        for layer, name in enumerate(["o1A", "o2A", "o3A"]):
            out[:, :, layer, rows, :] = r[name].reshape(B, D, HA, W)
        for layer, name in enumerate(["o1B", "o2B", "o3B"]):
            t = r[name].reshape(B, HB, W, D).transpose(0, 3, 1, 2)
            out[:, :, layer, rowsB, :] = t
    return out, res


def kernel(alpha_imgs: np.ndarray) -> np.ndarray:
    out, _ = _run(alpha_imgs, trace=False)
    return out


# revision 13
# speedup vs baseline: 1.1847x; 1.1847x over previous
"""Trainium2 Bass kernel for nn_Alpha2Assoc: 3-layer alpha compositing
with exclusive cumprod over D=32 planes.

Math per pixel (d = plane index, vectors over d):
    vis_l = excl_cumprod(1 - a_l);  out_l = vis_l * occ_{l-1};
    occ_l = 1 - vis_l;              a_{l+1} = a_l * occ_l
Output: stack([out_1, out_2, out_3], axis=2) -> [B, D, 3, H, W]

v2 design (fp16 I/O + two-path hybrid, engine-balanced):
  - All HBM I/O in fp16 (host converts): halves DMA traffic vs f32.
    Input clamped to <= 1-2^-11 host-side so ln(1-a) stays finite.
  - Path A ((b,d) on partitions, HA rows/core): cumprod in log space.
    Ln on ScalarE (affine absorbs 1-a), exclusive cumsum via TensorE
    f32r matmul with block-diag strictly-triangular 0/1 matrix,
    Exp(PSUM)->fp16 on ScalarE.  occ/a-recursion + out-muls on DVE
    (fp16: tensor_scalar 4x, tensor_tensor 2x).  ScalarE ~11.2us/tile.
  - Path B (transposed: pixel-sites on partitions, d contiguous in
    free dim, HB rows/core): DVE-only.  Segmented inclusive cumprod
    via tensor_tensor_scan(state = max(mask,state)*t) where mask=1 at
    d=0 cols resets each 32-group; scan writes at +1 col offset so
    consumers read the *exclusive* product at aligned offsets; a
    strided memset sets group-boundary cols to 1.  ~20.8us/tile, all
    on the otherwise-idle DVE.
  - Split HA/HB balances ScalarE (A only) against DVE (A muls + all
    of B).  B sub-stages (one per layer) are drip-fed between A
    pipeline stages.
"""

import math
from collections import deque

import numpy as np

import concourse.bass as bass
import concourse.tile as tile
from concourse import bacc, mybir
from concourse._compat import with_exitstack
from concourse.bass_utils import run_bass_kernel_spmd

# --- Pin Ln/Exp to the one table set containing both -------------------
_orig_get_activation_tables = bacc.get_activation_tables


def _pinned_get_activation_tables(arch):
    tables = _orig_get_activation_tables(arch)
    keep = {mybir.ActivationFunctionType.Ln, mybir.ActivationFunctionType.Exp}
    return {
        name: (fns if name == "natural_log_exp_and_others" else fns - keep)
        for name, fns in tables.items()
    }


bacc.get_activation_tables = _pinned_get_activation_tables

# --- Enable walrus LDWEIGHTS dedup (tri stationary identical across MMs)
import concourse.bass_utils as _bu

_orig_run_command = _bu.run_command


def _ldwopt_run_command(cmd, **kw):
    cmd = [c.replace("--enable-ldw-opt=false", "--enable-ldw-opt=true")
           if isinstance(c, str) else c for c in cmd]
    return _orig_run_command(cmd, **kw)


_bu.run_command = _ldwopt_run_command

B, D, H, W = 4, 32, 512, 512
P = 128
N_CORES = 8
H_SH = H // N_CORES                 # 64 rows per core
HA = 51                             # rows/core via log-space path A
HB = H_SH - HA                      # rows/core via DVE-scan path B
CA = HA * W                         # A cols per partition ((b,d) layout)
SITES_B = B * HB * W // P           # pixel-sites per partition (B layout)
CB = SITES_B * D                    # B cols per partition
TILE_N = 2048
MM_CHUNK = 512                      # f32r moving-operand max per matmul
PS_N = 2048                         # PSUM chunk (4 banks)


def _tiles(total, n):
    offs, sizes, o = [], [], 0
    while o < total:
        s = min(n, total - o)
        offs.append(o)
        sizes.append(s)
        o += s
    return list(zip(offs, sizes))


A_TILES = _tiles(CA, TILE_N)
B_TILES = _tiles(CB, TILE_N)

F16 = mybir.dt.float16
F32 = mybir.dt.float32
F32R = mybir.dt.float32r
AF = mybir.ActivationFunctionType
OP = mybir.AluOpType

_COMPILED = {}


def _tri_matrix() -> np.ndarray:
    """lhsT[k, m] = 1 iff same b-block and d_k < d_m (exclusive cumsum)."""
    k = np.arange(P)
    m = np.arange(P)
    same_b = (k[:, None] // D) == (m[None, :] // D)
    lower = (k[:, None] % D) < (m[None, :] % D)
    return (same_b & lower).astype(np.float32)


@with_exitstack
def _alpha_kernel(ctx, tc, aps):
    nc = tc.nc
    const_pool = ctx.enter_context(tc.tile_pool(name="const", bufs=1))
    # Warm the Ln/Exp act-table immediately (table load ~1.3us) so the
    # first real Ln doesn't serialize behind it.
    dummy = const_pool.tile([P, 8], F32)
    nc.vector.memset(dummy[:], 0.5)
    dummy_o = const_pool.tile([P, 8], F32)
    nc.scalar.activation(dummy_o[:], dummy[:], AF.Ln, bias=1.0, scale=-1.0)
    tri = const_pool.tile([P, P], F32R)
    msk = const_pool.tile([P, TILE_N], F16)

    # Path-A pools
    aA_pool = ctx.enter_context(tc.tile_pool(name="aA", bufs=5))
    u_pool = ctx.enter_context(tc.tile_pool(name="u", bufs=3))
    v_pool = ctx.enter_context(tc.tile_pool(name="v", bufs=8))
    occ_pool = ctx.enter_context(tc.tile_pool(name="occ", bufs=5))
    s_pool = ctx.enter_context(tc.tile_pool(name="s", bufs=5))
    o_pool = ctx.enter_context(tc.tile_pool(name="o", bufs=3))
    psum_pool = ctx.enter_context(tc.tile_pool(name="ps", bufs=2, space="PSUM"))
    # Path-B pools
    aB_pool = ctx.enter_context(tc.tile_pool(name="aB", bufs=2))
    t_pool = ctx.enter_context(tc.tile_pool(name="t", bufs=2))
    vb_pool = ctx.enter_context(tc.tile_pool(name="vb", bufs=3))
    occB_pool = ctx.enter_context(tc.tile_pool(name="occB", bufs=4))
    sB_pool = ctx.enter_context(tc.tile_pool(name="sB", bufs=4))
    oB_pool = ctx.enter_context(tc.tile_pool(name="oB", bufs=3))

    def cumsum_mm(u, n):
        # f32r single-pass matmuls into PSUM; the Exp is emitted later
        # (after the other layers' Lns) so ScalarE never idles on MM
        # latency.
        pss = []
        off = 0
        while off < n:
            psn = min(PS_N, n - off)
            ps = psum_pool.tile([P, psn], F32, tag="ps")
            for j0 in range(0, psn, MM_CHUNK):
                mc = min(MM_CHUNK, psn - j0)
                nc.tensor.matmul(
                    ps[:, bass.ds(j0, mc)],
                    tri[:],
                    u[:, bass.ds(off + j0, mc)],
                    start=True,
                    stop=True,
                )
            pss.append((off, psn, ps))
            off += psn
        return pss

    def cumsum_exp(pss, vis):
        for off, psn, ps in pss:
            nc.scalar.activation(
                vis[:, bass.ds(off, psn)], ps[:, bass.ds(0, psn)], AF.Exp
            )

    # ---- Path A stages (pipelined across tiles) ----
    stA = {}
    stA_pre = {}

    def prefetch_a(i):
        if i < len(A_TILES) and i not in stA_pre:
            off, n = A_TILES[i]
            a1 = aA_pool.tile([P, n], F16, tag="aA")
            nc.sync.dma_start(a1[:], aps["aA"][:, bass.ds(off, n)])
            stA_pre[i] = a1

    def stage_a1(i):
        off, n = A_TILES[i]
        sl = bass.ds(off, n)
        a1 = stA_pre.pop(i)
        u1 = u_pool.tile([P, n], F32R, tag="u")
        nc.scalar.activation(u1[:], a1[:], AF.Ln, bias=1.0, scale=-1.0)
        v1 = v_pool.tile([P, n], F16, tag="v")
        cumsum_exp(u1, v1, n)
        nc.sync.dma_start(aps["o1A"][:, sl], v1[:])
        stA[i] = {"a1": a1, "v1": v1}

    def stage_a2v(i):
        # Vector half, issued one iteration before the ScalarE consumer so
        # B-scan jitter in the DVE queue can't stall Ln2.
        off, n = A_TILES[i]
        d = stA[i]
        occ1 = occ_pool.tile([P, n], F16, tag="occ")
        nc.vector.tensor_scalar(occ1[:], d["v1"][:], -1.0, 1.0, OP.mult, OP.add)
        a2 = s_pool.tile([P, n], F16, tag="s")
        nc.vector.tensor_mul(a2[:], d["a1"][:], occ1[:])
        d.update({"occ1": occ1, "a2": a2})

    def stage_a2s(i):
        off, n = A_TILES[i]
        d = stA[i]
        u2 = u_pool.tile([P, n], F32R, tag="u")
        nc.scalar.activation(u2[:], d["a2"][:], AF.Ln, bias=1.0, scale=-1.0)
        v2 = v_pool.tile([P, n], F16, tag="v")
        cumsum_exp(u2, v2, n)
        d["v2"] = v2

    def stage_a3v(i):
        off, n = A_TILES[i]
        sl = bass.ds(off, n)
        d = stA[i]
        o2 = o_pool.tile([P, n], F16, tag="o")
        nc.vector.tensor_mul(o2[:], d["v2"][:], d["occ1"][:])
        nc.sync.dma_start(aps["o2A"][:, sl], o2[:])
        occ2 = occ_pool.tile([P, n], F16, tag="occ")
        nc.vector.tensor_scalar(occ2[:], d["v2"][:], -1.0, 1.0, OP.mult, OP.add)
        a3 = s_pool.tile([P, n], F16, tag="s")
        nc.vector.tensor_mul(a3[:], d["a2"][:], occ2[:])
        d.update({"occ2": occ2, "a3": a3})

    def stage_a3s(i):
        off, n = A_TILES[i]
        d = stA[i]
        u3 = u_pool.tile([P, n], F32R, tag="u")
        nc.scalar.activation(u3[:], d["a3"][:], AF.Ln, bias=1.0, scale=-1.0)
        v3 = v_pool.tile([P, n], F16, tag="v")
        cumsum_exp(u3, v3, n)
        d["v3"] = v3

    def stage_a4(i):
        off, n = A_TILES[i]
        sl = bass.ds(off, n)
        d = stA[i]
        o3 = o_pool.tile([P, n], F16, tag="o")
        nc.vector.tensor_mul(o3[:], d["v3"][:], d["occ2"][:])
        nc.sync.dma_start(aps["o3A"][:, sl], o3[:])
        del stA[i]

    # ---- Path B ops (DVE-only), emitted as fine-grained closures so they
    # slot between A stages without head-of-line blocking the A-path DVE
    # ops that ScalarE waits on. Input DMAs are prefetched ~1 tile ahead.
    stB = {}

    def scan_layer(tag, j, t, n):
        """inclusive segmented cumprod of t -> vE ([P, n]: exclusive
        product per col after boundary memset; group cols == 1)."""
        vbuf = vb_pool.tile([P, n + 32], F16, tag="vb")
        nc.vector.tensor_tensor_scan(
            vbuf[:, bass.ds(1, n)], msk[:, bass.ds(0, n)], t[:],
            0.0, OP.max, OP.mult,
        )
        ve = vbuf[:, bass.ds(0, n)]
        gcols = ve.rearrange("p (g d) -> p g d", d=32)[:, :, bass.ds(0, 1)]
        nc.vector.memset(gcols, 1.0)
        stB[j][tag] = ve

    def b_ops_for_tile(j):
        off, n = B_TILES[j]
        sl = bass.ds(off, n)
        d = stB.setdefault(j, {})

        def dma_in():
            a1 = aB_pool.tile([P, n], F16, tag="aB")
            nc.sync.dma_start(a1[:], aps["aB"][:, sl])
            d["a1"] = a1

        def t1():
            t = t_pool.tile([P, n], F16, tag="t")
            nc.vector.tensor_scalar(t[:], d["a1"][:], -1.0, 1.0,
                                    OP.mult, OP.add)
            d["t"] = t

        def occ_a(v_tag, a_tag, occ_tag, a_out_tag):
            occ = occB_pool.tile([P, n], F16, tag="occB")
            nc.vector.tensor_scalar(occ[:], d[v_tag], -1.0, 1.0,
                                    OP.mult, OP.add)
            a_n = sB_pool.tile([P, n], F16, tag="sB")
            nc.vector.tensor_mul(a_n[:], d[a_tag][:], occ[:])
            d[occ_tag], d[a_out_tag] = occ, a_n

        def t_next(a_tag):
            t = t_pool.tile([P, n], F16, tag="t")
            nc.vector.tensor_scalar(t[:], d[a_tag][:], -1.0, 1.0,
                                    OP.mult, OP.add)
            d["t"] = t

        def out_mul(v_tag, occ_tag, out_name):
            o = oB_pool.tile([P, n], F16, tag="oB")
            nc.vector.tensor_mul(o[:], d[v_tag], d[occ_tag][:])
            nc.sync.dma_start(aps[out_name][:, sl], o[:])

        return [
            dma_in,
            t1,
            lambda: scan_layer("ve1", j, d["t"], n),
            lambda: nc.sync.dma_start(aps["o1B"][:, sl], d["ve1"]),
            lambda: occ_a("ve1", "a1", "occ1", "a2"),
            lambda: t_next("a2"),
            lambda: scan_layer("ve2", j, d["t"], n),
            lambda: out_mul("ve2", "occ1", "o2B"),
            lambda: occ_a("ve2", "a2", "occ2", "a3"),
            lambda: t_next("a3"),
            lambda: scan_layer("ve3", j, d["t"], n),
            lambda: out_mul("ve3", "occ2", "o3B"),
            lambda: stB.pop(j),
        ]

    # flatten, hoisting each tile's dma_in 4 ops before the end of the
    # previous tile's ops (prefetch)
    b_flat = []
    for j in range(len(B_TILES)):
        ops = b_ops_for_tile(j)
        if j == 0:
            b_flat += ops
        else:
            dma, rest = ops[0], ops[1:]
            b_flat = b_flat[:-3] + [dma] + b_flat[-3:] + rest
    b_work = deque(b_flat)

    # ---- interleaved schedule ----
    # Stage offsets give every cross-engine dependency >= 1 full iteration
    # of slack: v1(i)->a2v at i+1 ->Ln2 at i+2; v2(i)@i+2 ->a3v at i+3
    # ->Ln3 at i+4; out3 at i+5.
    nA = len(A_TILES)
    n_iters = nA + 5
    quota = len(b_work) / (2 * (n_iters - 1))
    state = {"credit": 0.0}

    def drain_b(mult=1.0):
        state["credit"] += quota * mult
        while b_work and state["credit"] >= 1.0:
            b_work.popleft()()
            state["credit"] -= 1.0

    prefetch_a(0)
    drain_b(1.0)  # issues the aB(0) input DMA early
    nc.sync.dma_start(tri[:], aps["tri"][:, :])
    nc.sync.dma_start(msk[:], aps["msk"][:, :])
    drain_b(2.0)
    for k in range(n_iters):
        prefetch_a(k + 1)
        if k < nA:
            stage_a1(k)
        if 1 <= k - 1 + 1 and k - 1 < nA and k >= 1:
            stage_a2v(k - 1)
        if 2 <= k and k - 2 < nA:
            stage_a2s(k - 2)
        if 3 <= k and k - 3 < nA:
            stage_a3v(k - 3)
        drain_b()
        if 4 <= k and k - 4 < nA:
            stage_a3s(k - 4)
        if 5 <= k and k - 5 < nA:
            stage_a4(k - 5)
        drain_b()
    while b_work:
        b_work.popleft()()


def _build():
    nc = bacc.Bacc("TRN2", target_bir_lowering=False, debug=False,
                   num_devices=N_CORES)
    aps = {
        "aA": nc.dram_tensor("aA", [P, CA], F16, kind="ExternalInput").ap(),
        "aB": nc.dram_tensor("aB", [P, CB], F16, kind="ExternalInput").ap(),
        "tri": nc.dram_tensor("tri", [P, P], F32R, kind="ExternalInput").ap(),
        "msk": nc.dram_tensor("msk", [P, TILE_N], F16, kind="ExternalInput").ap(),
    }
    for name, cols in [("o1A", CA), ("o2A", CA), ("o3A", CA),
                       ("o1B", CB), ("o2B", CB), ("o3B", CB)]:
        aps[name] = nc.dram_tensor(name, [P, cols], F16,
                                   kind="ExternalOutput").ap()
    with tile.TileContext(nc) as tc:
        _alpha_kernel(tc, aps, )
    nc.compile()
    return nc


def _get_nc():
    if "nc" not in _COMPILED:
        _COMPILED["nc"] = _build()
    return _COMPILED["nc"]


def _run(alpha_imgs: np.ndarray, trace: bool = False):
    nc = _get_nc()
    tri = _tri_matrix()
    msk = np.zeros((P, TILE_N), np.float16)
    msk[:, ::32] = 1.0
    # fp16 with clamp so ln(1-a) stays finite (a==1.0 after fp16 rounding
    # would give -inf and 0*inf=NaN in the cumsum matmul)
    a16 = np.minimum(np.asarray(alpha_imgs, dtype=np.float16),
                     np.float16(1.0 - 2.0 ** -11))
    in_maps = []
    for c in range(N_CORES):
        blk = a16[:, :, 0, c * H_SH:(c + 1) * H_SH, :]   # [B, D, 64, W]
        aA = np.ascontiguousarray(blk[:, :, :HA, :]).reshape(P, CA)
        aB = np.ascontiguousarray(
            blk[:, :, HA:, :].transpose(0, 2, 3, 1)       # [B, HB, W, D]
        ).reshape(P, CB)
        in_maps.append({"aA": aA, "aB": aB, "tri": tri, "msk": msk})
    res = None
    backoffs = [3.0, 10.0, 30.0, 60.0]
    for attempt in range(len(backoffs) + 1):
        try:
            res = run_bass_kernel_spmd(
                nc, in_maps, core_ids=list(range(N_CORES)), trace=trace
            )
            break
        except Exception:
            if attempt == len(backoffs):
                raise
            import time

            time.sleep(backoffs[attempt])
    out = np.empty((B, D, 3, H, W), dtype=np.float32)
    for c in range(N_CORES):
        r = res.results[c]
        rows = slice(c * H_SH, c * H_SH + HA)
        rowsB = slice(c * H_SH + HA, (c + 1) * H_SH)
        for layer, name in enumerate(["o1A", "o2A", "o3A"]):
            out[:, :, layer, rows, :] = r[name].reshape(B, D, HA, W)
        for layer, name in enumerate(["o1B", "o2B", "o3B"]):
            t = r[name].reshape(B, HB, W, D).transpose(0, 3, 1, 2)
            out[:, :, layer, rowsB, :] = t
    return out, res


def kernel(alpha_imgs: np.ndarray) -> np.ndarray:
    out, _ = _run(alpha_imgs, trace=False)
    return out
